# revision 17
# baseline (speedup 1.0000x reference)
"""MADPSNet MoE-routing kernel for 8 Trainium2 NeuronCores.

The reference computes every expert on the full stacked input and then
gathers one expert per agent.  The routing indices (laac_shallow /
laac_deep) are host-visible numpy values, so we do the routing on the
host: per agent we select the 4 weight matrices of its chosen experts
and run only the selected chain

    x[2048,256] @ W1[256,512] -> relu -> @ W2[512,256] -> relu
                -> @ W3[256,512] -> relu -> @ W4[512,128] (+bias)

One agent per NeuronCore (A == 8 == n_cores), no collectives.

Layout: feature-major on chip (features on the 128 partitions, batch on
the free dim), everything bf16 except the fp32 PSUM accumulators (the
harness tolerance is 2e-2; bf16 end-to-end lands ~1e-3).  bf16 halves
the HBM traffic and runs matmuls at full PE rate with fast weight load,
vs the ~1.27x slower fp32 HIGH-mode pairs the fp32 path emits.

Adjacent output chunks (m, m+1) of one 512-column batch tile
accumulate into a two-bank [128,1024] PSUM pair from a 4-deep
rotation, drained to SBUF by ONE 1024-col ACT/DVE op (strictly
alternating engines) right after the second group closes -- legal
because the zero-bias drain is m-agnostic (with biases it falls back
to two 512-col ops).  Activations are stored bt-major so the pair
drain is one contiguous write.  Halved consumer-op count keeps both
engines ~65% busy and the write-after-read slack on bank reuse at
~1.5us, so the in-order PE queue never waits.  Layers are emitted
sequentially (a bt+2*li wavefront interleave measured slower; so did
all-single-bank tiles, whose 8-per-block drain bursts overload the
two consumer engines).

ALL supply DMAs ride the single sync HWDGE queue in compute-need
order -- the ring drains FIFO, so the L1 critical path (x bt0 / w1,
split into 128KB k-halves consumed by a k-outer first pass) gets all
16 SDMA engines first and each later transfer lands just ahead of its
consumer.  Splitting across queues makes the SDMA engines round-robin
between rings at packet granularity, which measurably delays the
critical transfers.  Warm-up matmuls on a zeroed scratch tile keep the
PE busy from the end of the framework preamble so the HAM clock gate
opens (1.2 -> 2.4 GHz) just as the first data lands.  The final batch
tile is computed as two 256-col groups in separate banks so its
drain->out-DMA chain is half as deep.  The kernel returns out^T
[128, 2048] bf16 per core; the host transposes and upcasts.
"""

import os

import numpy as np

import concourse.bass as bass
import concourse.mybir as mybir
from concourse import bacc
from concourse.bass_utils import run_bass_kernel_spmd
from concourse.tile import TileContext

A, B, S = 8, 2048, 256
H1, H2, D1, D2 = 512, 256, 512, 128
P = 128
BT = 512            # batch tile (psum bank: 512 fp32)
NBT = B // BT
NBP = NBT // 2      # batch super-tiles (pairs)

_DT_MAP = {
    "f32": mybir.dt.float32,
    "f32r": mybir.dt.float32r,
    "bf16": mybir.dt.bfloat16,
}

# layer: (k_chunks, m_chunks, bias col offset, relu?)
_LAYERS = [
    (S // P, H1 // P, 0, True),    # L1: 256 -> 512
    (H1 // P, H2 // P, 4, True),   # L2: 512 -> 256
    (H2 // P, D1 // P, 6, True),   # L3: 256 -> 512
    (D1 // P, D2 // P, 10, False), # L4: 512 -> 128
]


def _build(
    dt_name: str, add_bias: bool, warm: int, paird: bool, swdge: bool
) -> bass.Bass:
    dt = _DT_MAP[dt_name]
    f32 = mybir.dt.float32
    nc = bacc.Bacc(None, target_bir_lowering=False, debug=False)

    kx = S // P
    x_d = nc.dram_tensor("x", [P, kx * B], dt, kind="ExternalInput")
    w_ds = [
        nc.dram_tensor("w1", [P, (S // P) * H1], dt, kind="ExternalInput"),
        nc.dram_tensor("w2", [P, (H1 // P) * H2], dt, kind="ExternalInput"),
        nc.dram_tensor("w3", [P, (H2 // P) * D1], dt, kind="ExternalInput"),
        nc.dram_tensor("w4", [P, (D1 // P) * D2], dt, kind="ExternalInput"),
    ]
    b_d = (
        nc.dram_tensor("bias", [P, 11], f32, kind="ExternalInput")
        if add_bias
        else None
    )
    out_d = nc.dram_tensor("out", [D2, B], dt, kind="ExternalOutput")

    with TileContext(nc) as tc:
        with (
            tc.tile_pool(name="persist", bufs=1) as pp,
            tc.tile_pool(name="psum", bufs=3, space="PSUM") as psp,
        ):
            xt = pp.tile([P, kx * B], dt, tag="xt", name="xt")
            wts = [
                pp.tile(
                    [P, w_ds[i].shape[1]], dt, tag=f"w{i}", name=f"w{i}_sb"
                )
                for i in range(4)
            ]
            bti = (
                pp.tile([P, 11], f32, tag="bias", name="bias_sb")
                if add_bias
                else None
            )
            scr = (
                pp.tile([P, 2], f32, tag="scr", name="scr") if add_bias else None
            )
            # activations, bt-major: col = (bt*mc + m)*BT + b (adjacent
            # m-chunks of one batch tile are contiguous, so a two-bank
            # PSUM pair drains as one 1024-col write)
            acts = [
                pp.tile([P, n * B], dt, tag=f"a{li}", name=f"a{li}")
                for li, n in [(1, H1 // P), (2, H2 // P), (3, D1 // P)]
            ]
            ot = pp.tile([P, B], dt, tag="ot", name="ot")

            # ---- PE warm-up FIRST.  All engines clear the framework entry
            # barrier together at ~7.0us, and the first supply transfer
            # completes at ~10.0us (0.65us descriptor-gen + ~1.1us DMA
            # pipeline latency + ~0.6us stream, times two transfers).  The
            # warm-up chain must keep the PE *continuously* busy across
            # that whole window -- any idle gap resets the HAM activity
            # window and postpones the 1.2->2.4 GHz un-gate (measured: a
            # 1.5us gap pushed the gate from 11.0us to 15.2us).  Bridge
            # with a few 427ns cold 512-row matmuls, then 107ns 128-row
            # ones so the in-order PE queue frees right as data lands.
            wsb = pp.tile([P, BT], dt, tag="wsb", name="wsb")
            wps = psp.tile([P, 2 * BT], f32, tag="pp", bufs=4, name="wps")
            w512 = int(os.environ.get("MADPS_W512", "5"))
            w128 = int(os.environ.get("MADPS_W128", "5"))
            if w512 or w128:
                nc.gpsimd.memset(wsb[:], 0.0)
                for _ in range(w512):
                    nc.tensor.matmul(
                        wps[:, 0:BT], wsb[:, 0:P], wsb[:],
                        start=True, stop=True,
                    )
                for _ in range(w128):
                    nc.tensor.matmul(
                        wps[:, 0:P], wsb[:, 0:P], wsb[:, 0:P],
                        start=True, stop=True,
                    )

            # ---- input DMAs.  The two transfers the very first matmul
            # pass needs (x bt0 k0-half, w1 k0-half) are BOTH issued on the
            # scalar queue, back to back: scalar leaves the framework
            # preamble ~1.0us before sync, and one ring keeps them FIFO at
            # full bandwidth ahead of everything else.  ALL other supply
            # transfers stay on the single sync HWDGE queue in compute-need
            # order.  (Spreading them over scalar+gpsimd+sync measured
            # first-chunk completion at 12.2us instead of ~7.5us: the 16
            # SDMA engines round-robin across active rings at packet
            # granularity, so the critical chunks lost their priority.)
            def dma_x(eng, bt):
                sl = slice(bt * kx * BT, (bt + 1) * kx * BT)
                eng.dma_start(xt[:, sl], x_d[:, sl])

            def dma_half(dst, src, h, eng=None):
                n = dst.shape[1] // 2
                sl = slice(h * n, (h + 1) * n)
                (eng or nc.sync).dma_start(dst[:, sl], src[:, sl])

            edma = _feat("MADPS_EDMA", "0")
            dma_half(xt[:, 0 : kx * BT], x_d[:, 0 : kx * BT], 0,
                     nc.scalar if edma else None)
            dma_half(wts[0], w_ds[0], 0, nc.scalar if edma else None)
            dma_half(xt[:, 0 : kx * BT], x_d[:, 0 : kx * BT], 1)
            dma_half(wts[0], w_ds[0], 1)
            # all x tiles split into k-halves so each lands ~0.65us after
            # the previous issue; weights follow the x stream they gate
            for bt in range(1, NBT):
                sl = slice(bt * kx * BT, (bt + 1) * kx * BT)
                dma_half(xt[:, sl], x_d[:, sl], 0)
                dma_half(xt[:, sl], x_d[:, sl], 1)
            nc.sync.dma_start(wts[1][:], w_ds[1][:])
            nc.sync.dma_start(wts[2][:], w_ds[2][:])
            nc.sync.dma_start(wts[3][:], w_ds[3][:])
            if add_bias:
                nc.scalar.dma_start(bti[:], b_d[:])

            if add_bias:
                # advance ACT/DVE engine clocks past the bias DMA so the
                # real post-matmul ops carry a single (PE) wait each.
                nc.scalar.copy(scr[:, 0:1], bti[:, 0:1])
                nc.vector.tensor_copy(scr[:, 1:2], bti[:, 0:1])

            # ---- the 4-layer chain over 2 batch super-tiles, bf16
            # matmuls accumulating into [128,1024] two-bank PSUM pairs.
            def rhs(li, k, bt):
                if li == 0:
                    return xt[:, (bt * kx + k) * BT : (bt * kx + k + 1) * BT]
                src = acts[li - 1]
                kc = _LAYERS[li][0]
                return src[:, (bt * kc + k) * BT : (bt * kc + k + 1) * BT]

            ndrain = 0

            def drain(ps_ap, dst, boff_m, relu):
                """PSUM -> SBUF with bias+relu, alternating ACT/DVE."""
                nonlocal ndrain
                use_act = ndrain % 2 == 1
                ndrain += 1
                if add_bias:
                    bias_ap = bti[:, boff_m : boff_m + 1]
                    if use_act:
                        func = (
                            mybir.ActivationFunctionType.Relu
                            if relu
                            else mybir.ActivationFunctionType.Identity
                        )
                        nc.scalar.activation(dst, ps_ap, func, bias=bias_ap)
                    elif relu:
                        nc.vector.tensor_scalar(
                            dst, ps_ap, bias_ap, 0.0,
                            mybir.AluOpType.add, mybir.AluOpType.max,
                        )
                    else:
                        nc.vector.tensor_scalar_add(dst, ps_ap, bias_ap)
                elif use_act:
                    func = (
                        mybir.ActivationFunctionType.Relu
                        if relu
                        else mybir.ActivationFunctionType.Copy
                    )
                    nc.scalar.activation(dst, ps_ap, func)
                elif relu:
                    nc.vector.tensor_scalar_max(dst, ps_ap, 0.0)
                else:
                    nc.vector.tensor_copy(dst, ps_ap)

            def emit_block(li, bt):
                kc, mc, boff, relu = _LAYERS[li]
                wt = wts[li]

                def wchunk(k, m):
                    return wt[:, (k * mc + m) * P : (k * mc + m + 1) * P]

                def pair_tile(name):
                    # adjacent m-chunks of one batch tile accumulate into
                    # a two-bank pair from a 4-deep rotation; the pair
                    # drains as ONE 1024-col op right after its second
                    # group closes (no bias => the op is m-agnostic),
                    # halving consumer-op count and keeping ~1.5us of
                    # WAR slack before bank reuse
                    return psp.tile([P, 2 * BT], f32, tag="pp", bufs=4,
                                    name=name)

                def drain_pair(ps_ap, mp):
                    dst = acts[li][
                        :, (bt * mc + 2 * mp) * BT : (bt * mc + 2 * mp + 2) * BT
                    ]
                    if add_bias:
                        # per-m bias scalars differ across the halves:
                        # drain them as two 512-col ops
                        for mi in range(2):
                            drain(
                                ps_ap[:, mi * BT : (mi + 1) * BT],
                                dst[:, mi * BT : (mi + 1) * BT],
                                boff + 2 * mp + mi, relu,
                            )
                    else:
                        drain(ps_ap, dst, boff, relu)

                if li < 3:
                    np_ = mc // 2
                    prs = [pair_tile(f"pp{li}_{bt}_{mp}") for mp in range(np_)]
                    if li == 0 and bt <= 1:
                        # k-outer: each pass starts as soon as its 128KB
                        # x/w1 DMA half lands
                        for k in range(kc):
                            for m in range(mc):
                                nc.tensor.matmul(
                                    prs[m // 2][:, (m % 2) * BT : (m % 2 + 1) * BT],
                                    wchunk(k, m), rhs(0, k, bt),
                                    start=(k == 0), stop=(k == kc - 1),
                                )
                        for mp in range(np_):
                            drain_pair(prs[mp][:], mp)
                    else:
                        for m in range(mc):
                            for k in range(kc):
                                nc.tensor.matmul(
                                    prs[m // 2][:, (m % 2) * BT : (m % 2 + 1) * BT],
                                    wchunk(k, m), rhs(li, k, bt),
                                    start=(k == 0), stop=(k == kc - 1),
                                )
                            if m % 2 == 1:
                                drain_pair(prs[m // 2][:], m // 2)
                    return

                # L4 (mc == 1): single-bank groups, drain + ship per batch
                # tile; the final tile as two 256-col groups in separate
                # banks so the tail chain is half as deep.  (A 4x128-col
                # fan-out over 3 engine queues measured 2us WORSE: four
                # 0.65us descriptor-gen issues serialize on the engines and
                # the last one lands deep in the exit ceremony.)
                def do_group(sl, eng):
                    sng = psp.tile([P, BT], f32, tag="pp", bufs=4,
                                   name=f"ps3_{bt}_{sl.start}")
                    dst_ap = sng[:, sl]
                    for k in range(kc):
                        nc.tensor.matmul(
                            dst_ap, wchunk(k, 0), rhs(li, k, bt)[:, sl],
                            start=(k == 0), stop=(k == kc - 1),
                        )
                    lo, hi = bt * BT + sl.start, bt * BT + sl.stop
                    d = ot[:, lo:hi]
                    drain(dst_ap, d, boff, relu)
                    eng.dma_start(out_d[:, lo:hi], d)

                if bt == NBT - 1:
                    h = BT // 2
                    do_group(slice(0, h), nc.sync)
                    do_group(slice(h, BT), nc.scalar)
                else:
                    do_group(slice(0, BT),
                             nc.sync if bt % 2 == 0 else nc.scalar)

            for li in range(4):
                for bt in range(NBT):
                    emit_block(li, bt)

            # ---- trailing dummy matmuls: keep the PE busy through the
            # drain/out-DMA tail so the HAM clock gate is still open
            # (K=8/8) when the runtime's per-engine semaphore-reset
            # epilogue starts -- the Tensor engine's ~52-reset sweep is
            # the epilogue critical path.
            tailwarm = int(os.environ.get("MADPS_TAILWARM", "0"))
            for _ in range(tailwarm):
                # 512-row dummies (216ns warm); reads of the never-written
                # wsb[:, 128:512] region are harmless -- wps is scratch
                nc.tensor.matmul(
                    wps[:, 0:BT], wsb[:, 0:P], wsb[:],
                    start=True, stop=True,
                )
    nc.compile()
    return nc


_BUILT: dict[tuple, bass.Bass] = {}


def _cfg():
    dt_name = os.environ.get("MADPS_DT", "bf16")
    warm = int(os.environ.get("MADPS_WARM", "12"))
    return dt_name, warm


def _feat(name: str, default: str = "1") -> bool:
    return os.environ.get(name, default) == "1"


def _get_nc(dt_name: str, add_bias: bool, warm: int) -> bass.Bass:
    paird = _feat("MADPS_PAIRD")
    swdge = _feat("MADPS_SWDGE")
    key = (dt_name, add_bias, warm, paird, swdge)
    if key not in _BUILT:
        _BUILT[key] = _build(dt_name, add_bias, warm, paird, swdge)
    return _BUILT[key]


def _np_dt(dt_name: str):
    if dt_name == "bf16":
        import ml_dtypes

        return ml_dtypes.bfloat16
    return np.float32


def _packw(w: np.ndarray, np_dt) -> np.ndarray:
    """[K, M] -> [128, (K/128)*M], k-chunk-major: col (k*mc + m)*128 + j."""
    k, m = w.shape
    kc = k // P
    return np.ascontiguousarray(
        w.reshape(kc, P, m).transpose(1, 0, 2).reshape(P, -1).astype(np_dt)
    )


def _prepare(inputs, dt_name):
    """Returns (add_bias, in_maps) for run_bass_kernel_spmd."""
    np_dt = _np_dt(dt_name)

    x = np.asarray(inputs["inputs"], dtype=np.float32)
    sel_s = np.asarray(inputs["laac_shallow"]).reshape(-1).astype(np.int64)
    sel_d = np.asarray(inputs["laac_deep"]).reshape(-1).astype(np.int64)
    Ws1 = np.asarray(inputs["Ws1"], dtype=np.float32)
    Ws2 = np.asarray(inputs["Ws2"], dtype=np.float32)
    Wd1 = np.asarray(inputs["Wd1"], dtype=np.float32)
    Wd2 = np.asarray(inputs["Wd2"], dtype=np.float32)
    bs1 = np.asarray(inputs["bs1"], dtype=np.float32)
    bs2 = np.asarray(inputs["bs2"], dtype=np.float32)
    bd1 = np.asarray(inputs["bd1"], dtype=np.float32)
    bd2 = np.asarray(inputs["bd2"], dtype=np.float32)

    add_bias = any(
        float(np.abs(b).max()) != 0.0 for b in (bs1, bs2, bd1, bd2)
    )

    in_maps = []
    for a in range(A):
        es, ed = int(sel_s[a]), int(sel_d[a])
        # bt-major packing: col = bt*(S//P)*BT + k*BT + b
        xp = np.ascontiguousarray(
            x[a]
            .reshape(NBT, BT, S // P, P)
            .transpose(3, 0, 2, 1)
            .reshape(P, -1)
            .astype(np_dt)
        )
        m = {
            "x": xp,
            "w1": _packw(Ws1[es], np_dt),
            "w2": _packw(Ws2[es], np_dt),
            "w3": _packw(Wd1[ed], np_dt),
            "w4": _packw(Wd2[ed], np_dt),
        }
        if add_bias:
            bias_cols = np.concatenate([bs1[es], bs2[es], bd1[ed], bd2[ed]])
            m["bias"] = np.ascontiguousarray(
                bias_cols.reshape(11, P).T, dtype=np.float32
            )
        in_maps.append(m)
    return add_bias, in_maps


def kernel(**inputs) -> np.ndarray:
    dt_name, warm = _cfg()
    add_bias, in_maps = _prepare(inputs, dt_name)
    nc = _get_nc(dt_name, add_bias, warm)
    res = run_bass_kernel_spmd(nc, in_maps, list(range(A)))
    out = np.stack(
        [np.asarray(res.results[a]["out"]).astype(np.float32).T for a in range(A)]
    )
    return np.ascontiguousarray(out)



# revision 18
# speedup vs baseline: 1.0546x; 1.0546x over previous
"""MADPSNet MoE-routing kernel for 8 Trainium2 NeuronCores.

The reference computes every expert on the full stacked input and then
gathers one expert per agent.  The routing indices (laac_shallow /
laac_deep) are host-visible numpy values, so we do the routing on the
host: per agent we select the 4 weight matrices of its chosen experts
and run only the selected chain

    x[2048,256] @ W1[256,512] -> relu -> @ W2[512,256] -> relu
                -> @ W3[256,512] -> relu -> @ W4[512,128] (+bias)

One agent per NeuronCore (A == 8 == n_cores), no collectives.

Layout: feature-major on chip (features on the 128 partitions, batch on
the free dim), everything bf16 except the fp32 PSUM accumulators (the
harness tolerance is 2e-2; bf16 end-to-end lands ~1e-3).  bf16 halves
the HBM traffic and runs matmuls at full PE rate with fast weight load,
vs the ~1.27x slower fp32 HIGH-mode pairs the fp32 path emits.

Adjacent output chunks (m, m+1) of one 512-column batch tile
accumulate into a two-bank [128,1024] PSUM pair from a 4-deep
rotation, drained to SBUF by ONE 1024-col ACT/DVE op (strictly
alternating engines) right after the second group closes -- legal
because the zero-bias drain is m-agnostic (with biases it falls back
to two 512-col ops).  Activations are stored bt-major so the pair
drain is one contiguous write.  Halved consumer-op count keeps both
engines ~65% busy and the write-after-read slack on bank reuse at
~1.5us, so the in-order PE queue never waits.  Layers are emitted
sequentially (a bt+2*li wavefront interleave measured slower; so did
all-single-bank tiles, whose 8-per-block drain bursts overload the
two consumer engines).

ALL supply DMAs ride the single sync HWDGE queue in compute-need
order -- the ring drains FIFO, so the L1 critical path (x bt0 / w1,
split into 128KB k-halves consumed by a k-outer first pass) gets all
16 SDMA engines first and each later transfer lands just ahead of its
consumer.  Splitting across queues makes the SDMA engines round-robin
between rings at packet granularity, which measurably delays the
critical transfers.  Warm-up matmuls on a zeroed scratch tile keep the
PE busy from the end of the framework preamble so the HAM clock gate
opens (1.2 -> 2.4 GHz) just as the first data lands.  The final batch
tile is computed as two 256-col groups in separate banks so its
drain->out-DMA chain is half as deep.  The kernel returns out^T
[128, 2048] bf16 per core; the host transposes and upcasts.
"""

import os

import numpy as np

import concourse.bass as bass
import concourse.mybir as mybir
from concourse import bacc
from concourse.bass_utils import run_bass_kernel_spmd
from concourse.tile import TileContext

A, B, S = 8, 2048, 256
H1, H2, D1, D2 = 512, 256, 512, 128
P = 128
BT = 512            # batch tile (psum bank: 512 fp32)
NBT = B // BT
NBP = NBT // 2      # batch super-tiles (pairs)

_DT_MAP = {
    "f32": mybir.dt.float32,
    "f32r": mybir.dt.float32r,
    "bf16": mybir.dt.bfloat16,
}

# layer: (k_chunks, m_chunks, bias col offset, relu?)
_LAYERS = [
    (S // P, H1 // P, 0, True),    # L1: 256 -> 512
    (H1 // P, H2 // P, 4, True),   # L2: 512 -> 256
    (H2 // P, D1 // P, 6, True),   # L3: 256 -> 512
    (D1 // P, D2 // P, 10, False), # L4: 512 -> 128
]


def _build(
    dt_name: str, add_bias: bool, warm: int, paird: bool, swdge: bool
) -> bass.Bass:
    dt = _DT_MAP[dt_name]
    f32 = mybir.dt.float32
    nc = bacc.Bacc(None, target_bir_lowering=False, debug=False)

    kx = S // P
    x_d = nc.dram_tensor("x", [P, kx * B], dt, kind="ExternalInput")
    w_ds = [
        nc.dram_tensor("w1", [P, (S // P) * H1], dt, kind="ExternalInput"),
        nc.dram_tensor("w2", [P, (H1 // P) * H2], dt, kind="ExternalInput"),
        nc.dram_tensor("w3", [P, (H2 // P) * D1], dt, kind="ExternalInput"),
        nc.dram_tensor("w4", [P, (D1 // P) * D2], dt, kind="ExternalInput"),
    ]
    b_d = (
        nc.dram_tensor("bias", [P, 11], f32, kind="ExternalInput")
        if add_bias
        else None
    )
    out_d = nc.dram_tensor("out", [D2, B], dt, kind="ExternalOutput")

    with TileContext(nc) as tc:
        with (
            tc.tile_pool(name="persist", bufs=1) as pp,
            tc.tile_pool(name="psum", bufs=3, space="PSUM") as psp,
        ):
            xt = pp.tile([P, kx * B], dt, tag="xt", name="xt")
            wts = [
                pp.tile(
                    [P, w_ds[i].shape[1]], dt, tag=f"w{i}", name=f"w{i}_sb"
                )
                for i in range(4)
            ]
            bti = (
                pp.tile([P, 11], f32, tag="bias", name="bias_sb")
                if add_bias
                else None
            )
            scr = (
                pp.tile([P, 2], f32, tag="scr", name="scr") if add_bias else None
            )
            # activations, bt-major: col = (bt*mc + m)*BT + b (adjacent
            # m-chunks of one batch tile are contiguous, so a two-bank
            # PSUM pair drains as one 1024-col write)
            acts = [
                pp.tile([P, n * B], dt, tag=f"a{li}", name=f"a{li}")
                for li, n in [(1, H1 // P), (2, H2 // P), (3, D1 // P)]
            ]
            ot = pp.tile([P, B], dt, tag="ot", name="ot")

            # ---- PE warm-up FIRST.  All engines clear the framework entry
            # barrier together at ~7.0us, and the first supply transfer
            # completes at ~10.0us (0.65us descriptor-gen + ~1.1us DMA
            # pipeline latency + ~0.6us stream, times two transfers).  The
            # warm-up chain must keep the PE *continuously* busy across
            # that whole window -- any idle gap resets the HAM activity
            # window and postpones the 1.2->2.4 GHz un-gate (measured: a
            # 1.5us gap pushed the gate from 11.0us to 15.2us).  Bridge
            # with a few 427ns cold 512-row matmuls, then 107ns 128-row
            # ones so the in-order PE queue frees right as data lands.
            # One long accumulation chain of 128-row matmuls: no start/stop
            # between them, so there is no ~200ns PSUM-retire gap per
            # matmul and each costs only ~107ns cold.  Sized so the queue
            # drains roughly when the first supply data lands (~10.3us,
            # +-1us run-to-run from cross-core HBM contention).
            wsb = pp.tile([P, BT], dt, tag="wsb", name="wsb")
            wps = psp.tile([P, 2 * BT], f32, tag="pp", bufs=4, name="wps")
            w128 = int(os.environ.get("MADPS_W128", "28"))
            if w128:
                nc.gpsimd.memset(wsb[:, 0:P], 0.0)
                for i in range(w128):
                    nc.tensor.matmul(
                        wps[:, 0:P], wsb[:, 0:P], wsb[:, 0:P],
                        start=(i == 0), stop=(i == w128 - 1),
                    )

            # ---- input DMAs.  The two transfers the very first matmul
            # pass needs (x bt0 k0-half, w1 k0-half) are BOTH issued on the
            # scalar queue, back to back: scalar leaves the framework
            # preamble ~1.0us before sync, and one ring keeps them FIFO at
            # full bandwidth ahead of everything else.  ALL other supply
            # transfers stay on the single sync HWDGE queue in compute-need
            # order.  (Spreading them over scalar+gpsimd+sync measured
            # first-chunk completion at 12.2us instead of ~7.5us: the 16
            # SDMA engines round-robin across active rings at packet
            # granularity, so the critical chunks lost their priority.)
            def dma_x(eng, bt):
                sl = slice(bt * kx * BT, (bt + 1) * kx * BT)
                eng.dma_start(xt[:, sl], x_d[:, sl])

            def dma_half(dst, src, h, eng=None):
                n = dst.shape[1] // 2
                sl = slice(h * n, (h + 1) * n)
                (eng or nc.sync).dma_start(dst[:, sl], src[:, sl])

            edma = _feat("MADPS_EDMA", "0")
            dma_half(xt[:, 0 : kx * BT], x_d[:, 0 : kx * BT], 0,
                     nc.scalar if edma else None)
            dma_half(wts[0], w_ds[0], 0, nc.scalar if edma else None)
            dma_half(xt[:, 0 : kx * BT], x_d[:, 0 : kx * BT], 1)
            dma_half(wts[0], w_ds[0], 1)
            # all x tiles split into k-halves so each lands ~0.65us after
            # the previous issue; weights follow the x stream they gate
            for bt in range(1, NBT):
                sl = slice(bt * kx * BT, (bt + 1) * kx * BT)
                dma_half(xt[:, sl], x_d[:, sl], 0)
                dma_half(xt[:, sl], x_d[:, sl], 1)
            nc.sync.dma_start(wts[1][:], w_ds[1][:])
            nc.sync.dma_start(wts[2][:], w_ds[2][:])
            nc.sync.dma_start(wts[3][:], w_ds[3][:])
            if add_bias:
                nc.scalar.dma_start(bti[:], b_d[:])

            if add_bias:
                # advance ACT/DVE engine clocks past the bias DMA so the
                # real post-matmul ops carry a single (PE) wait each.
                nc.scalar.copy(scr[:, 0:1], bti[:, 0:1])
                nc.vector.tensor_copy(scr[:, 1:2], bti[:, 0:1])

            # ---- the 4-layer chain over 2 batch super-tiles, bf16
            # matmuls accumulating into [128,1024] two-bank PSUM pairs.
            def rhs(li, k, bt):
                if li == 0:
                    return xt[:, (bt * kx + k) * BT : (bt * kx + k + 1) * BT]
                src = acts[li - 1]
                kc = _LAYERS[li][0]
                return src[:, (bt * kc + k) * BT : (bt * kc + k + 1) * BT]

            ndrain = 0

            def drain(ps_ap, dst, boff_m, relu):
                """PSUM -> SBUF with bias+relu, alternating ACT/DVE."""
                nonlocal ndrain
                use_act = ndrain % 2 == 1
                ndrain += 1
                if add_bias:
                    bias_ap = bti[:, boff_m : boff_m + 1]
                    if use_act:
                        func = (
                            mybir.ActivationFunctionType.Relu
                            if relu
                            else mybir.ActivationFunctionType.Identity
                        )
                        nc.scalar.activation(dst, ps_ap, func, bias=bias_ap)
                    elif relu:
                        nc.vector.tensor_scalar(
                            dst, ps_ap, bias_ap, 0.0,
                            mybir.AluOpType.add, mybir.AluOpType.max,
                        )
                    else:
                        nc.vector.tensor_scalar_add(dst, ps_ap, bias_ap)
                elif use_act:
                    func = (
                        mybir.ActivationFunctionType.Relu
                        if relu
                        else mybir.ActivationFunctionType.Copy
                    )
                    nc.scalar.activation(dst, ps_ap, func)
                elif relu:
                    nc.vector.tensor_scalar_max(dst, ps_ap, 0.0)
                else:
                    nc.vector.tensor_copy(dst, ps_ap)

            def emit_block(li, bt):
                kc, mc, boff, relu = _LAYERS[li]
                wt = wts[li]

                def wchunk(k, m):
                    return wt[:, (k * mc + m) * P : (k * mc + m + 1) * P]

                def pair_tile(name):
                    # adjacent m-chunks of one batch tile accumulate into
                    # a two-bank pair from a 4-deep rotation; the pair
                    # drains as ONE 1024-col op right after its second
                    # group closes (no bias => the op is m-agnostic),
                    # halving consumer-op count and keeping ~1.5us of
                    # WAR slack before bank reuse
                    return psp.tile([P, 2 * BT], f32, tag="pp", bufs=4,
                                    name=name)

                def drain_pair(ps_ap, mp):
                    dst = acts[li][
                        :, (bt * mc + 2 * mp) * BT : (bt * mc + 2 * mp + 2) * BT
                    ]
                    if add_bias:
                        # per-m bias scalars differ across the halves:
                        # drain them as two 512-col ops
                        for mi in range(2):
                            drain(
                                ps_ap[:, mi * BT : (mi + 1) * BT],
                                dst[:, mi * BT : (mi + 1) * BT],
                                boff + 2 * mp + mi, relu,
                            )
                    else:
                        drain(ps_ap, dst, boff, relu)

                if li < 3:
                    np_ = mc // 2
                    prs = [pair_tile(f"pp{li}_{bt}_{mp}") for mp in range(np_)]
                    if li == 0 and bt <= 1:
                        # k-outer: each pass starts as soon as its 128KB
                        # x/w1 DMA half lands
                        for k in range(kc):
                            for m in range(mc):
                                nc.tensor.matmul(
                                    prs[m // 2][:, (m % 2) * BT : (m % 2 + 1) * BT],
                                    wchunk(k, m), rhs(0, k, bt),
                                    start=(k == 0), stop=(k == kc - 1),
                                )
                        for mp in range(np_):
                            drain_pair(prs[mp][:], mp)
                    else:
                        for m in range(mc):
                            for k in range(kc):
                                nc.tensor.matmul(
                                    prs[m // 2][:, (m % 2) * BT : (m % 2 + 1) * BT],
                                    wchunk(k, m), rhs(li, k, bt),
                                    start=(k == 0), stop=(k == kc - 1),
                                )
                            if m % 2 == 1:
                                drain_pair(prs[m // 2][:], m // 2)
                    return

                # L4 (mc == 1): single-bank groups, drain + ship per batch
                # tile; the final tile as two 256-col groups in separate
                # banks so the tail chain is half as deep.  (A 4x128-col
                # fan-out over 3 engine queues measured 2us WORSE: four
                # 0.65us descriptor-gen issues serialize on the engines and
                # the last one lands deep in the exit ceremony.)
                def do_group(sl, eng):
                    sng = psp.tile([P, BT], f32, tag="pp", bufs=4,
                                   name=f"ps3_{bt}_{sl.start}")
                    dst_ap = sng[:, sl]
                    for k in range(kc):
                        nc.tensor.matmul(
                            dst_ap, wchunk(k, 0), rhs(li, k, bt)[:, sl],
                            start=(k == 0), stop=(k == kc - 1),
                        )
                    lo, hi = bt * BT + sl.start, bt * BT + sl.stop
                    d = ot[:, lo:hi]
                    drain(dst_ap, d, boff, relu)
                    eng.dma_start(out_d[:, lo:hi], d)

                if bt == NBT - 1:
                    h = BT // 2
                    do_group(slice(0, h), nc.sync)
                    do_group(slice(h, BT), nc.scalar)
                else:
                    do_group(slice(0, BT),
                             nc.sync if bt % 2 == 0 else nc.scalar)

            for li in range(4):
                for bt in range(NBT):
                    emit_block(li, bt)

            # ---- trailing dummy matmuls: keep the PE busy through the
            # drain/out-DMA tail so the HAM clock gate is still open
            # (K=8/8) when the runtime's per-engine semaphore-reset
            # epilogue starts -- the Tensor engine's ~52-reset sweep is
            # the epilogue critical path.
            tailwarm = int(os.environ.get("MADPS_TAILWARM", "0"))
            for _ in range(tailwarm):
                # 512-row dummies (216ns warm); reads of the never-written
                # wsb[:, 128:512] region are harmless -- wps is scratch
                nc.tensor.matmul(
                    wps[:, 0:BT], wsb[:, 0:P], wsb[:],
                    start=True, stop=True,
                )
    nc.compile()
    return nc


_BUILT: dict[tuple, bass.Bass] = {}


def _cfg():
    dt_name = os.environ.get("MADPS_DT", "bf16")
    warm = int(os.environ.get("MADPS_WARM", "12"))
    return dt_name, warm


def _feat(name: str, default: str = "1") -> bool:
    return os.environ.get(name, default) == "1"


def _get_nc(dt_name: str, add_bias: bool, warm: int) -> bass.Bass:
    paird = _feat("MADPS_PAIRD")
    swdge = _feat("MADPS_SWDGE")
    key = (dt_name, add_bias, warm, paird, swdge)
    if key not in _BUILT:
        _BUILT[key] = _build(dt_name, add_bias, warm, paird, swdge)
    return _BUILT[key]


def _np_dt(dt_name: str):
    if dt_name == "bf16":
        import ml_dtypes

        return ml_dtypes.bfloat16
    return np.float32


def _packw(w: np.ndarray, np_dt) -> np.ndarray:
    """[K, M] -> [128, (K/128)*M], k-chunk-major: col (k*mc + m)*128 + j."""
    k, m = w.shape
    kc = k // P
    return np.ascontiguousarray(
        w.reshape(kc, P, m).transpose(1, 0, 2).reshape(P, -1).astype(np_dt)
    )


def _prepare(inputs, dt_name):
    """Returns (add_bias, in_maps) for run_bass_kernel_spmd."""
    np_dt = _np_dt(dt_name)

    x = np.asarray(inputs["inputs"], dtype=np.float32)
    sel_s = np.asarray(inputs["laac_shallow"]).reshape(-1).astype(np.int64)
    sel_d = np.asarray(inputs["laac_deep"]).reshape(-1).astype(np.int64)
    Ws1 = np.asarray(inputs["Ws1"], dtype=np.float32)
    Ws2 = np.asarray(inputs["Ws2"], dtype=np.float32)
    Wd1 = np.asarray(inputs["Wd1"], dtype=np.float32)
    Wd2 = np.asarray(inputs["Wd2"], dtype=np.float32)
    bs1 = np.asarray(inputs["bs1"], dtype=np.float32)
    bs2 = np.asarray(inputs["bs2"], dtype=np.float32)
    bd1 = np.asarray(inputs["bd1"], dtype=np.float32)
    bd2 = np.asarray(inputs["bd2"], dtype=np.float32)

    add_bias = any(
        float(np.abs(b).max()) != 0.0 for b in (bs1, bs2, bd1, bd2)
    )

    in_maps = []
    for a in range(A):
        es, ed = int(sel_s[a]), int(sel_d[a])
        # bt-major packing: col = bt*(S//P)*BT + k*BT + b
        xp = np.ascontiguousarray(
            x[a]
            .reshape(NBT, BT, S // P, P)
            .transpose(3, 0, 2, 1)
            .reshape(P, -1)
            .astype(np_dt)
        )
        m = {
            "x": xp,
            "w1": _packw(Ws1[es], np_dt),
            "w2": _packw(Ws2[es], np_dt),
            "w3": _packw(Wd1[ed], np_dt),
            "w4": _packw(Wd2[ed], np_dt),
        }
        if add_bias:
            bias_cols = np.concatenate([bs1[es], bs2[es], bd1[ed], bd2[ed]])
            m["bias"] = np.ascontiguousarray(
                bias_cols.reshape(11, P).T, dtype=np.float32
            )
        in_maps.append(m)
    return add_bias, in_maps


def kernel(**inputs) -> np.ndarray:
    dt_name, warm = _cfg()
    add_bias, in_maps = _prepare(inputs, dt_name)
    nc = _get_nc(dt_name, add_bias, warm)
    res = run_bass_kernel_spmd(nc, in_maps, list(range(A)))
    out = np.stack(
        [np.asarray(res.results[a]["out"]).astype(np.float32).T for a in range(A)]
    )
    return np.ascontiguousarray(out)

